# revision 103
# baseline (speedup 1.0000x reference)
"""Trainium2 Bass kernel for nn_CAAN_84112639525649 (CAAN dense transformer).

Shapes: B=16, N=512, D_IN=256, D=64, E=32, MAXD=50.
Sharding: data-parallel over batch, 2 batches per core on 8 cores.

Host precomputes every parameter-only transform (an inference server would
cache these with the weights):
  - f[d] = sigmoid(relu(Eemb[d] @ Wr1 + br1) @ Wr2), d in [0, 50]; the pairwise
    rank bias rel[i,j] = f[clip(|r_i-r_j|,0,50)] collapses to this table.
  - g'[k] = f[min(|k-511|,50)] - f[50] and the three Toeplitz blocks
    GT_d[k,m] = g'[511 + 128d + m - k], d in {-1,0,1}: with one-hot
    RT[v][p,j] = [r_j == 128v+p], the bias is
      rel^T - f50 = sum_v RT[v]-select of T1, T1_v = sum_w GT_{v-w} RT_w.
  - M1T = ([Wq;bq][Wk;bk]^T)^T / 8 (score bilinear form, pre-transposed for
    the [key j, query i] attention layout).
  - Ws1' = diag(ln_g) Ws1, bs1' = ln_b @ Ws1 + bs1 (LN affine folded).

On-device everything runs bf16 operands / f32 PSUM. Attention is computed
directly in transposed [j, i] layout (scores^T, rel^T=rel): no A-transposes,
and the softmax denominator falls out of a ones-column in V; ao accumulates
per key-chunk so there is no separate AV barrier. The unnormalized-P product
(rel + f50) * S reads both PSUM banks directly (no score drain). 1/Z is
broadcast across partitions with a rank-1 PE outer product. The scalar engine
uses only {copy, relu, exp} plus one batched Sqrt per batch, giving three
act-table loads total. A run of identity transposes at kernel start keeps the
PE busy through the DMA phase so real matmuls hit the ramped 2.4 GHz clock.
"""

import sys
import os

for _p in ("/opt/trn_rl_repo",):
    if os.path.isdir(_p) and _p not in sys.path:
        sys.path.insert(0, _p)

import numpy as np
import ml_dtypes
from contextlib import ExitStack

import concourse.bass as bass
import concourse.tile as tile
from concourse import bacc, mybir
from concourse.bass import ts

N_CORES = 8
B = 16
PB = B // N_CORES  # batches per core
N = 512
D_IN = 256
D = 64
E = 32
MAXD = 50
LN_EPS = 1e-5
FILLERS = 8  # PE warm-up matmuls bridging the input-DMA phase

f32 = mybir.dt.float32
f32r = mybir.dt.float32r
bf16 = mybir.dt.bfloat16
f16 = mybir.dt.float16
i32 = mybir.dt.int32
AF = mybir.ActivationFunctionType
OP = mybir.AluOpType

# ---- packed weight layouts -------------------------------------------------
# wp32: f32 [128, F32], entries name -> (row, col, rows, cols)
WP32 = {
    "bp":    (0, 0, 64, 1),       # bias for xp, column
    "bvb":   (0, 1, 128, 66),     # bv broadcast cols 0..63, col64=1.0, col65=0
    "bf1":   (0, 67, 128, 1),
    "bf2b":  (0, 68, 128, 64),
    "bs1p":  (0, 132, 32, 1),
    "nbs2b": (0, 133, 128, 1),    # -bs2 replicated
    "f50c":  (0, 134, 128, 1),
    "epsc":  (0, 135, 128, 1),
    "iotaf": (0, 136, 128, 4),
    "ones1": (0, 140, 1, 128),    # rank-1 broadcast row for the 1/Z outer
}
F32C = 268
# wp16: bf16 [128, F16]
WP16 = {
    "Gm":    (0, 0, 128, 128),    # GT_{-1}
    "G0":    (0, 128, 128, 128),
    "Gp":    (0, 256, 128, 128),
    "Wp0":   (0, 384, 128, 64),
    "Wp1":   (0, 448, 128, 64),
    "Wf2":   (0, 512, 128, 64),
    "Wf1":   (0, 576, 64, 128),
    "Ws1p":  (0, 704, 64, 32),
    "Ws2":   (0, 736, 32, 1),
    "Wv66":  (0, 737, 65, 66),
    "M1":    (0, 803, 65, 66),
    "bf1r":  (64, 869, 1, 128),
    "bf2r":  (64, 997, 1, 64),
}
F16C = 1061


def build_nc():
    nc = bacc.Bacc(
        "TRN2",
        target_bir_lowering=False,
        debug=False,
        enable_asserts=False,
        num_devices=N_CORES,
    )

    # ---- DRAM I/O ----
    x_d = nc.dram_tensor("x", (PB, N, D_IN), bf16, kind="ExternalInput")
    w32_d = nc.dram_tensor("wp32", (128, F32C), f32, kind="ExternalInput")
    w16_d = nc.dram_tensor("wp16", (128, F16C), bf16, kind="ExternalInput")
    rr_d = nc.dram_tensor("rr", (1, PB * N), f16, kind="ExternalInput")
    ri_d = nc.dram_tensor("ri", (128, 32 * PB), mybir.dt.int16,
                          kind="ExternalInput")
    out_d = nc.dram_tensor("out", (PB, N), f32, kind="ExternalOutput")
    t1_d = nc.dram_tensor("t1hbm", (N, N), bf16, kind="Internal")

    with ExitStack() as ctx:
        tc = ctx.enter_context(tile.TileContext(nc))
        cp = ctx.enter_context(tc.tile_pool(name="consts", bufs=1))
        wk = ctx.enter_context(tc.tile_pool(name="work", bufs=1))
        ps = ctx.enter_context(tc.tile_pool(name="ps", bufs=5, space="PSUM"))

        PS_BUFS = {"mm": 2, "io": 2, "sm": 2, "tp": 1, "ao": 1}

        def psum(shape, tag="mm", dtype=f32, bufs=None):
            if bufs is None:
                bufs = PS_BUFS[tag]
            return ps.tile(shape, dtype, tag=tag, name="pst", bufs=bufs)

        # ============ input DMAs (order = transfer order; DMA is serial) ====
        rr_sb = cp.tile([1, PB * N], f16, tag="rr")
        nc.sync.dma_start(rr_sb[:, :], rr_d.ap())
        ri_sb = cp.tile([128, 32 * PB], mybir.dt.int16, tag="ri")
        nc.sync.dma_start(ri_sb[:, :], ri_d.ap())
        wp16 = cp.tile([128, F16C], bf16, tag="wp16")
        nc.sync.dma_start(wp16[:, :], w16_d.ap())
        wp32 = cp.tile([128, F32C], f32, tag="wp32")
        nc.sync.dma_start(wp32[:, :], w32_d.ap())
        # x: one 3D DMA per batch -> xall[b] [128 tok, 4 tchunk, 256 din]
        xall = []
        for b in range(PB):
            xt = wk.tile([128, 4, D_IN], bf16, tag=f"xall{b}", name="xall")
            nc.sync.dma_start(
                xt[:, :, :],
                bass.AP(tensor=x_d, offset=b * N * D_IN,
                        ap=[[D_IN, 128], [128 * D_IN, 4], [1, D_IN]]),
            )
            xall.append(xt)

        def w16(name):
            row, col, rows, cols = WP16[name]
            return wp16[row : row + rows, col : col + cols]

        def w32s(name):
            row, col, rows, cols = WP32[name]
            return wp32[row : row + rows, col : col + cols]

        GT = {-1: w16("Gm"), 0: w16("G0"), 1: w16("Gp")}
        iotaf = w32s("iotaf")

        bp_c = w32s("bp")
        bvb = w32s("bvb")
        bf1_c = w32s("bf1")
        bf2b = w32s("bf2b")
        bs1p_c = w32s("bs1p")
        nbs2b = w32s("nbs2b")
        f50c = w32s("f50c")
        epsc = w32s("epsc")
        ones1 = w32s("ones1")

        # identity built on-chip (no DMA dependency) so PE warm-up can start
        # immediately; wide filler matmuls (engine-bound, back-to-back) hold
        # the clock ramp through the DMA phase
        ident_io = cp.tile([128, 128], i32, tag="ident_io")
        nc.gpsimd.iota(ident_io[:, :], pattern=[[-1, 128]], base=0,
                       channel_multiplier=1)
        ident = cp.tile([128, 128], bf16, tag="ident")
        nc.vector.tensor_scalar(ident[:, :], ident_io[:, :], 0, None,
                                op0=OP.is_equal)
        fill_src = cp.tile([128, N], bf16, tag="fill_src")
        nc.gpsimd.memset(fill_src[:, :], 0.0)
        iota_i = cp.tile([128, 4], i32, tag="iota_i")
        nc.gpsimd.iota(iota_i[:, :], pattern=[[128, 4]], base=0,
                       channel_multiplier=1)
        iotac = cp.tile([128, 4], f32, tag="iotac")
        nc.gpsimd.tensor_copy(iotac[:, :], iota_i[:, :])
        dscr0 = cp.tile([1, 1], f32, tag="dscr0")
        nc.vector.memset(dscr0[:, :], 0.0)
        nc.scalar.activation(dscr0[:, :], dscr0[:, :], AF.Exp)
        xpT_t = []
        for b in range(PB):
            xpT = wk.tile([D + 1, N], bf16, tag=f"xpT{b}", name="xpT")
            nc.gpsimd.memset(xpT[D : D + 1, :], 1.0)
            xpT_t.append(xpT)
        for _ in range(FILLERS):
            fp = psum([128, N], tag="tp", dtype=f32)
            nc.tensor.matmul(fp[:, :], ident[:, :], fill_src[:, :],
                             start=True, stop=True)

        # ================= per-batch stages =================================
        st_ = [dict() for _ in range(PB)]

        def stage_onehot(b):
            S = st_[b]
            r_bc = wk.tile([128, N], f16, tag=f"r_bc{b}", name="rbc")
            nc.gpsimd.partition_broadcast(r_bc[:, :],
                                          rr_sb[0:1, b * N : (b + 1) * N])
            S["RT"] = []
            for v in range(4):
                rt = wk.tile([128, N], bf16, tag=f"RT{b}_{v}", name="rt")
                nc.vector.tensor_scalar(rt[:, :], r_bc[:, :],
                                        iotac[:, v : v + 1], None,
                                        op0=OP.is_equal)
                S["RT"].append(rt)

        T1_TAGS = ["mm", "mm", "ao", "tp"]

        def stage_t1(b):
            S = st_[b]
            S["T1"] = []
            for v in range(4):
                t1p = psum([128, N], tag=T1_TAGS[v])
                ws = [w for w in (v - 1, v, v + 1) if 0 <= w <= 3]
                for wi, w in enumerate(ws):
                    nc.tensor.matmul(t1p[:, :], GT[v - w], S["RT"][w][:, :],
                                     start=(wi == 0), stop=(wi == len(ws) - 1))
                t1s = wk.tile([128, N], bf16, tag=f"T1{b}_{v}",
                              name="t1s")
                if v % 2 == 0:
                    nc.vector.tensor_copy(t1s[:, :], t1p[:, :])
                else:
                    nc.scalar.copy(t1s[:, :], t1p[:, :])
                if b == 1:
                    nc.sync.dma_start(
                        bass.AP(tensor=t1_d, offset=128 * v * N,
                                ap=[[N, 128], [1, N]]),
                        t1s[:, :])
                S["T1"].append(t1s)
            if b == 1:
                relg = wk.tile([128, 4, N], bf16, tag=f"relg{b}",
                               name="relg")
                nc.gpsimd.dma_gather(
                    relg[:, :, :],
                    bass.AP(tensor=t1_d, offset=0, ap=[[N, N], [1, N]]),
                    ri_sb[:, 32 : 64],
                    num_idxs=N, num_idxs_reg=N, elem_size=N)
                S["relg"] = relg

        def stage_xp(b):
            S = st_[b]
            xT = [wk.tile([128, N], bf16, tag=f"xT{b}_{d}", name="xT")
                  for d in range(2)]
            for d in range(2):
                xT_ps = psum([128, N], tag="tp", dtype=bf16)
                for t in range(4):
                    nc.tensor.transpose(
                        xT_ps[:, ts(t, 128)], xall[b][:, t, ts(d, 128)],
                        ident)
                if d == 0:
                    nc.scalar.copy(xT[d][:, :], xT_ps[:, :])
                else:
                    nc.vector.tensor_copy(xT[d][:, :], xT_ps[:, :])
            xpT_ps = psum([D, N], tag="io")
            for d in range(2):
                nc.tensor.matmul(xpT_ps[:, :], w16(f"Wp{d}"), xT[d][:, :],
                                 start=(d == 0), stop=(d == 1))
            xpT = xpT_t[b]
            nc.scalar.copy(xpT[:D, :], xpT_ps[:, :])
            S["xpT"] = xpT

        def stage_uv(b):
            S = st_[b]
            xpT = S["xpT"]
            uT_ps = psum([D + 2, N], tag="io")
            nc.tensor.matmul(uT_ps[:, :], w16("M1"), xpT[: D + 1, :],
                             start=True, stop=True)
            uT = wk.tile([D + 1, N], bf16, tag=f"uT{b}", name="uT")
            nc.scalar.copy(uT[:, :], uT_ps[: D + 1, :])
            if b == 0:
                # anchored dummy: pins the exp-table reload here (~12us),
                # fully hidden behind the gather wait
                dscr2 = wk.tile([1, 1], f32, tag="dscr2", name="dscr2")
                nc.scalar.activation(dscr2[:, :], uT[0:1, 0:1], AF.Exp)
            S["uT"] = uT
            S["v_sb"] = []
            for c in range(4):
                v_ps = psum([128, D + 2], tag="sm")
                nc.tensor.matmul(v_ps[:, : D + 2], xpT[: D + 1, ts(c, 128)],
                                 w16("Wv66"), start=True, stop=True)
                vs = wk.tile([128, D + 2], bf16, tag=f"v{b}_{c}", name="vs")
                if c % 2 == 0:
                    nc.vector.tensor_copy(vs[:, : D + 2], v_ps[:, : D + 2])
                else:
                    nc.scalar.copy(vs[:, : D + 2], v_ps[:, : D + 2])
                S["v_sb"].append(vs)

        def stage_attn_mm(b, c):
            # c indexes the key chunk j; PT_c[j, i] = exp(S^T * rel^T)
            S = st_[b]
            sp = psum([128, N])
            nc.tensor.matmul(sp[:, :], S["uT"][: D + 1, ts(c, 128)],
                             S["xpT"][: D + 1, :], start=True, stop=True)
            p_st = wk.tile([128, N], bf16, tag=f"P{b}_{c}", name="pst")
            if b == 0:
                relp = psum([128, N], tag="tp")
                for v in range(4):
                    nc.tensor.matmul(relp[:, :], S["RT"][v][:, ts(c, 128)],
                                     S["T1"][v][:, :], start=(v == 0),
                                     stop=(v == 3))
                ssb = wk.tile([128, N], bf16, tag=f"S{b}_{c}", name="ssb")
                if c % 2 == 0:
                    nc.scalar.copy(ssb[:, :], sp[:, :])
                else:
                    nc.vector.tensor_copy(ssb[:, :], sp[:, :])
                nc.vector.scalar_tensor_tensor(p_st[:, :], relp[:, :],
                                               f50c[:, 0:1], ssb[:, :],
                                               op0=OP.add, op1=OP.mult)
            else:
                nc.vector.scalar_tensor_tensor(p_st[:, :],
                                               S["relg"][:, c, :],
                                               f50c[:, 0:1], sp[:, :],
                                               op0=OP.add, op1=OP.mult)
            nc.scalar.activation(p_st[:, :], p_st[:, :], AF.Exp)
            S.setdefault("PT", []).append(p_st)

        def stage_attn_ao(b, c):
            # emitted one chunk behind the mm stage so the PE never waits
            # on the exp of the chunk it just produced
            S = st_[b]
            if c == 0:
                S["aoT_ps"] = psum([D + 2, N], tag="ao")
            nc.tensor.matmul(S["aoT_ps"][:, :], S["v_sb"][c][:, :],
                             S["PT"][c][:, :], start=(c == 0), stop=(c == 3))

        def stage_av_fin(b):
            # drain unnormalized [ao; Z]; LN is scale-invariant so 1/Z is
            # never applied — Z rides along as Z*bf1 / Z*bf2 rank-1 updates
            S = st_[b]
            aou = wk.tile([D + 1, N], bf16, tag=f"aou{b}", name="aou")
            if b == 1:
                nc.scalar.copy(aou[:, :], S["aoT_ps"][: D + 1, :])
            else:
                nc.vector.tensor_copy(aou[:, :], S["aoT_ps"][: D + 1, :])
            S["aou"] = aou

        def stage_ffn_head(b, c):
            # per-i-chunk: h'' = Z*(h1 Wf2 + bf2), LN stats (Z divides out)
            S = st_[b]
            aou = S["aou"]
            if c == 0:
                S["h_all"] = wk.tile([128, 4, D], bf16, tag=f"h{b}",
                                     name="hsb")
                S["mv"] = wk.tile([128, 8], f32, tag=f"mv{b}", name="mv")
                S["h1T"] = wk.tile([2 * D, N], bf16, tag=f"h1T{b}",
                                   name="h1T")
                h1_ps = psum([2 * D, N], tag="io")
                nc.tensor.matmul(h1_ps[:, :], w16("Wf1"), aou[:D, :],
                                 start=True, stop=False)
                nc.tensor.matmul(h1_ps[:, :], w16("bf1r"),
                                 aou[D : D + 1, :], start=False, stop=True)
                if b == 0:
                    nc.vector.tensor_scalar(S["h1T"][:, :], h1_ps[:, :],
                                            0.0, None, op0=OP.max)
                else:
                    nc.scalar.activation(S["h1T"][:, :], h1_ps[:, :],
                                         AF.Relu)
            h_ps = psum([128, D], tag="sm")
            nc.tensor.matmul(h_ps[:, :], S["h1T"][:, ts(c, 128)],
                             w16("Wf2"), start=True, stop=False)
            nc.tensor.matmul(h_ps[:, :], aou[D : D + 1, ts(c, 128)],
                             w16("bf2r"), start=False, stop=True)
            nc.vector.tensor_copy(S["h_all"][:, c, :], h_ps[:, :])
            stats = wk.tile([128, 6], f32, tag=f"st{b}_{c}", name="sts")
            nc.vector.bn_stats(stats[:, :], S["h_all"][:, c, :])
            nc.vector.bn_aggr(S["mv"][:, 2 * c : 2 * c + 2], stats[:, :])

        def stage_rstd(b, dummy=False):
            S = st_[b]
            std4 = wk.tile([128, 4], f32, tag=f"std{b}", name="std")
            nc.scalar.activation(std4[:, :], S["mv"][:, 1::2], AF.Sqrt,
                                 bias=epsc[:, 0:1], scale=1.0)
            rstd = wk.tile([128, 4], f32, tag=f"rstd{b}", name="rstd")
            nc.vector.reciprocal(rstd[:, :], std4[:, :])
            S["rstd"] = rstd
            if dummy:
                # tiny sigmoid pulls the act-table switch off the output tail
                dscr = wk.tile([1, 1], f32, tag="dscr", name="dscr")
                nc.scalar.activation(dscr[:, :], epsc[0:1, 0:1], AF.Sigmoid)

        def stage_ffn_tail(b):
            S = st_[b]
            zT_ps = psum([D, N], tag="tp", dtype=bf16)
            s1T_ps = psum([32, N], tag="io")
            s1T = wk.tile([32, N], bf16, tag=f"s1T{b}", name="s1T")
            o_ps = psum([128, 4], tag="sm")
            zT = wk.tile([D, N], bf16, tag=f"zT{b}", name="zT")
            for c in range(4):
                z_sb = wk.tile([128, D], bf16, tag=f"z{b}_{c}", name="zsb")
                zeng = nc.gpsimd if b == 0 else nc.vector
                zeng.tensor_scalar(z_sb[:, :], S["h_all"][:, c, :],
                                   S["mv"][:, 2 * c : 2 * c + 1],
                                   S["rstd"][:, c : c + 1],
                                   op0=OP.subtract, op1=OP.mult)
                nc.tensor.transpose(zT_ps[:, ts(c, 128)], z_sb[:, :D],
                                    ident)
            nc.vector.tensor_copy(zT[:, :], zT_ps[:, :])
            nc.tensor.matmul(s1T_ps[:, :], w16("Ws1p"), zT[:, :],
                             start=True, stop=True)
            nc.vector.tensor_scalar(s1T[:, :], s1T_ps[:, :],
                                    bs1p_c[:32, 0:1], 0.0,
                                    op0=OP.add, op1=OP.max)
            for c in range(4):
                nc.tensor.matmul(o_ps[:, c : c + 1], s1T[:, ts(c, 128)],
                                 w16("Ws2"), start=True, stop=True)
            o_sb = wk.tile([128, 4], f32, tag=f"o{b}", name="osb")
            nc.scalar.activation(o_sb[:, :], o_ps[:, :], AF.Sigmoid,
                                 bias=nbs2b[:, 0:1], scale=1.0)
            # out[b, 128c + p] <- o_sb[p, c]
            nc.sync.dma_start(
                bass.AP(tensor=out_d, offset=b * N, ap=[[1, 128], [128, 4]]),
                o_sb[:, :])

        # interleaved emission: T1 work can start before x lands
        stage_onehot(0)
        stage_onehot(1)
        stage_t1(0)
        stage_t1(1)
        stage_xp(0)
        stage_uv(0)
        stage_xp(1)
        stage_uv(1)
        stage_attn_mm(0, 0)
        for c in range(1, 4):
            stage_attn_mm(0, c)
            stage_attn_ao(0, c - 1)
        stage_attn_mm(1, 0)
        stage_attn_ao(0, 3)
        stage_av_fin(0)
        for c in range(1, 4):
            stage_attn_mm(1, c)
            stage_attn_ao(1, c - 1)
            stage_ffn_head(0, c - 1)
        stage_attn_ao(1, 3)
        stage_av_fin(1)
        stage_ffn_head(0, 3)
        stage_rstd(0)
        for c in range(4):
            stage_ffn_head(1, c)
        stage_rstd(1, dummy=True)
        stage_ffn_tail(0)
        stage_ffn_tail(1)

    nc.compile()
    return nc


def _f(a):
    return np.asarray(a, np.float32)


def _sigmoid(x):
    return 1.0 / (1.0 + np.exp(-x))


def _pack(inputs):
    """Host-side parameter preprocessing (weight-only transforms)."""
    w32 = np.zeros((128, F32C), np.float32)
    w16 = np.zeros((128, F16C), ml_dtypes.bfloat16)

    def put32(name, arr):
        row, col, rows, cols = WP32[name]
        w32[row : row + rows, col : col + cols] = _f(arr).reshape(rows, cols)

    def put16(name, arr):
        row, col, rows, cols = WP16[name]
        w16[row : row + rows, col : col + cols] = (
            _f(arr).reshape(rows, cols).astype(ml_dtypes.bfloat16))

    Wp = _f(inputs["Wp"])
    put16("Wp0", Wp[:128])
    put16("Wp1", Wp[128:])
    # transposed: the [j, i]-layout scores need sc[i, j] at position (j, i)
    Wqa = np.concatenate([_f(inputs["Wq"]), _f(inputs["bq"]).reshape(1, D)], 0)
    Wka = np.concatenate([_f(inputs["Wk"]), _f(inputs["bk"]).reshape(1, D)], 0)
    # absorb the xp bias: xpa_old = T @ [xp_raw; 1]
    T = np.eye(65, dtype=np.float32)
    T[:64, 64] = _f(inputs["bp"])
    M1 = np.zeros((65, 66), np.float32)
    M1[:, :65] = T.T @ ((Wka @ Wqa.T) / 8.0) @ T
    put16("M1", M1)
    Wv66 = np.zeros((65, 66), np.float32)
    Wv66[:64, :64] = _f(inputs["Wv"])
    Wv66[64, :64] = _f(inputs["bp"]) @ _f(inputs["Wv"]) + _f(inputs["bv"])
    Wv66[64, 64] = 1.0
    put16("Wv66", Wv66)
    put16("Wf1", inputs["Wf1"])
    put16("Wf2", inputs["Wf2"])
    put16("bf1r", _f(inputs["bf1"]).reshape(1, 2 * D))
    put16("bf2r", _f(inputs["bf2"]).reshape(1, D))
    # LN affine folded into the score head
    Ws1 = _f(inputs["Ws1"])
    put16("Ws1p", _f(inputs["ln_g"]).reshape(D, 1) * Ws1)
    put16("Ws2", inputs["Ws2"])
    # f table -> g' line -> Toeplitz blocks
    f = _sigmoid(
        np.maximum(_f(inputs["Eemb"]) @ _f(inputs["Wr1"])
                   + _f(inputs["br1"]), 0.0) @ _f(inputs["Wr2"]))[:, 0]
    f = f[: MAXD + 1]
    f50 = float(f[MAXD])
    k = np.arange(1023)
    gline = f[np.minimum(np.abs(k - 511), MAXD)] - f50
    for d, nm in ((-1, "Gm"), (0, "G0"), (1, "Gp")):
        kk = np.arange(128)
        mm = np.arange(128)
        idx = 511 + 128 * d + mm[None, :] - kk[:, None]
        put16(nm, np.where((idx >= 0) & (idx < 1023),
                           gline[np.clip(idx, 0, 1022)], 0.0))

    put32("iotaf",
          np.arange(128)[:, None] + 128 * np.arange(4)[None, :])
    put32("ones1", np.ones((1, 128), np.float32))
    put32("bp", inputs["bp"])
    bvb = np.zeros((128, 66), np.float32)
    bvb[:, :64] = _f(inputs["bv"])[None, :]
    bvb[:, 64] = 1.0
    put32("bvb", bvb)
    put32("bf1", _f(inputs["bf1"]).reshape(128, 1))
    put32("bf2b", np.broadcast_to(_f(inputs["bf2"]), (128, D)))
    put32("bs1p", _f(inputs["ln_b"]) @ Ws1 + _f(inputs["bs1"]))
    put32("nbs2b", np.full((128, 1), float(_f(inputs["bs2"])[0]), np.float32))
    put32("f50c", np.full((128, 1), f50, np.float32))
    put32("epsc", np.full((128, 1), LN_EPS, np.float32))
    return w32, w16


_NC_CACHE = {}


def _get_nc():
    if "nc" not in _NC_CACHE:
        _NC_CACHE["nc"] = build_nc()
    return _NC_CACHE["nc"]


def kernel(**inputs):
    from concourse.bass_utils import run_bass_kernel_spmd

    nc = _get_nc()

    x = np.ascontiguousarray(
        _f(inputs["x"]).astype(ml_dtypes.bfloat16))
    ri = np.asarray(inputs["price_rising_ranks"]).astype(np.int16)
    r = np.asarray(inputs["price_rising_ranks"]).astype(np.float16)
    assert x.shape == (B, N, D_IN)

    w32, w16 = _pack(inputs)
    in_maps = []
    for c in range(N_CORES):
        # gather idxs: idx k lives at [k % 16, k // 16], tiled to 128 rows
        rib = np.zeros((128, 32 * PB), np.int16)
        for b in range(PB):
            rb = ri[c * PB + b]
            wrap = rb.reshape(32, 16).T  # [16, 32]: [k%16, k//16]
            rib[:, 32 * b : 32 * b + 32] = np.tile(wrap, (8, 1))
        in_maps.append({
            "x": np.ascontiguousarray(x[c * PB : (c + 1) * PB]),
            "rr": np.ascontiguousarray(
                r[c * PB : (c + 1) * PB].reshape(1, PB * N)),
            "ri": rib,
            "wp32": w32,
            "wp16": w16,
        })

    res = run_bass_kernel_spmd(nc, in_maps, core_ids=list(range(N_CORES)))
    out = np.concatenate([res.results[c]["out"] for c in range(N_CORES)], axis=0)
    return out.astype(np.float32)
